# revision 1
# baseline (speedup 1.0000x reference)
"""Trainium2 Bass kernel for nn_MultiHeadAttention (B=2, S=2048, D=1024, H=16).

Reference semantics (note the *raw-view* head split):
    q = query @ Wq.T + bq                  # [B, S, D]
    q = q.reshape(B, H, S, DK)             # raw view: head h = rows [h*128,(h+1)*128) of q[b]
                                           #   viewed as [2048, 64]
    scores = q @ k.T / sqrt(DK), causal mask, softmax
    ctx    = softmax @ v                   # [B, H, S, DK]
    out    = ctx.transpose(0,2,1,3).reshape(B,S,D) @ Wo.T + bo

Sharding: 8 cores = 2 batches x 4 head-groups.  Core (b, g) owns heads
[4g, 4g+4) of batch b, i.e. rows [512g, 512g+512) of the QKV projections.
Each core computes its 4 heads' attention plus its partial contribution
C_heads @ Wo[:, head cols].T of the output projection; the host sums the 4
partials per batch and adds bo.

On-device dataflow per core (all fp32, matmuls in fp32r @ full rate):
  - q,k projections emit feature-major tiles [f,r]; a strided "scatter" copy
    builds per-head-pair tensors qT/kT [128=2 heads x 64 dk, 2048 positions].
  - v projection emits row-major [r,f]; a DMA reshape builds vh tiles
    [128 pos, 64 dk] (+ ones column for softmax denominators).
  - scores_T = kT.T @ qT per (k-tile 128, q-window 512), row-tiled so both
    heads of a pair run concurrently in the PE array.
  - exp on ACT, causal masking per diagonal tile, then ctx_T[dk(+denom), q]
    accumulates with M=65 matmuls (vh ones column -> row 64 = softmax denom).
  - normalize with reciprocal + gpsimd partition_broadcast, then the output
    projection out[s, o] = sum_pairs ctxT_pair.T @ WoT_pair.
"""

import os
import sys

import numpy as np

_TRN_REPO = "/opt/trn_rl_repo"
if _TRN_REPO not in sys.path:
    sys.path.insert(0, _TRN_REPO)

B, S, D, H = 2, 2048, 1024, 16
DK = D // H  # 64
N_CORES = 8
HEADS_PER_CORE = H // 4  # 4
ROWS_PER_CORE = HEADS_PER_CORE * (S // H)  # 512 rows of the projection output
QW = 512  # q-position window (psum free-dim)
KT = 128  # k-position tile


def _build_program(repeat=1, phases=3, ivl=True):
    import concourse.bass as bass
    import concourse.bacc as bacc
    import concourse.mybir as mybir
    from concourse.tile import TileContext
    from concourse import library_config

    f32 = mybir.dt.float32
    f32r = mybir.dt.float32r
    Exp = mybir.ActivationFunctionType.Exp
    Copy = mybir.ActivationFunctionType.Copy
    Identity = mybir.ActivationFunctionType.Identity
    MUL = mybir.AluOpType.mult
    ADD = mybir.AluOpType.add

    nc = bacc.Bacc("TRN2", target_bir_lowering=False, debug=False)

    # ---- DRAM parameters (host pre-tiled / pre-transposed) ----
    xq = nc.dram_tensor("xq", [8, 128, QW], f32r, kind="ExternalInput")
    xk = nc.dram_tensor("xk", [8, 128, QW], f32r, kind="ExternalInput")
    xv = nc.dram_tensor("xv", [8, 128, QW], f32r, kind="ExternalInput")
    wq = nc.dram_tensor("wq", [2, 8, 128, 512], f32r, kind="ExternalInput")
    wk = nc.dram_tensor("wk", [2, 8, 128, 512], f32r, kind="ExternalInput")
    wv = nc.dram_tensor("wv", [2, 8, 128, 512], f32r, kind="ExternalInput")
    wo = nc.dram_tensor("wo", [2, 128, 1024], f32r, kind="ExternalInput")
    bqd = nc.dram_tensor("bqd", [128, 16], f32, kind="ExternalInput")
    bkd = nc.dram_tensor("bkd", [128, 16], f32, kind="ExternalInput")
    bvr = nc.dram_tensor("bvr", [1, 1024], f32r, kind="ExternalInput")
    tri = nc.dram_tensor("tri", [128, 128], f32, kind="ExternalInput")
    ones128 = nc.dram_tensor("ones128", [1, 128], f32r, kind="ExternalInput")
    out = nc.dram_tensor("out", [S, D], f32, kind="ExternalOutput")

    with TileContext(nc) as tc:
      from contextlib import ExitStack
      with ExitStack() as stack:
        persist = stack.enter_context(tc.tile_pool(name="persist", bufs=1))
        vhp = stack.enter_context(tc.tile_pool(name="vhp", bufs=1))
        small = stack.enter_context(tc.tile_pool(name="small", bufs=4))
        xp = stack.enter_context(tc.tile_pool(name="xp", bufs=2))
        wp = stack.enter_context(tc.tile_pool(name="wp", bufs=2))
        vsb = stack.enter_context(tc.tile_pool(name="vsb", bufs=1))
        ptp = stack.enter_context(tc.tile_pool(name="ptp", bufs=4))
        wop = stack.enter_context(tc.tile_pool(name="wop", bufs=1))
        osb = stack.enter_context(tc.tile_pool(name="osb", bufs=3))
        for rep in range(repeat):
            # persistent tiles
            qpair = [persist.tile([128, S], f32r, tag=f"qpair{p}", name=f"qpair{p}") for p in range(2)]
            kpair = [persist.tile([128, S], f32r, tag=f"kpair{p}", name=f"kpair{p}") for p in range(2)]
            ctxT = [persist.tile([128, S], f32r, tag=f"ctxT{p}", name=f"ctxT{p}") for p in range(2)]
            tri01 = persist.tile([128, 128], f32, tag="tri01")
            bq_t = persist.tile([128, 16], f32, tag="bq_t")
            bk_t = persist.tile([128, 16], f32, tag="bk_t")
            bv_t = persist.tile([1, 1024], f32r, tag="bv_t")
            ones_row = persist.tile([1, 128], f32r, tag="ones_row")
            nc.sync.dma_start(out=tri01[:], in_=tri[:])
            nc.sync.dma_start(out=bq_t[:], in_=bqd[:])
            nc.sync.dma_start(out=bk_t[:], in_=bkd[:])
            nc.sync.dma_start(out=bv_t[:], in_=bvr[:])
            nc.sync.dma_start(out=ones_row[:], in_=ones128[:])


            # vh: one tile per head [128, 16*(DK+1)]; per ktile j cols
            # [j*65, j*65+64) = v data, col j*65+64 = ones (softmax denom)
            vh = [
                vhp.tile([128, 16 * (DK + 1)], f32r, tag=f"vh_{h}", name=f"vh_{h}")
                for h in range(4)
            ]

            # ---------------- Phase 1: projections ----------------
            with (
                tc.tile_pool(name=f"pps{rep}", bufs=3, space="PSUM") as pps,
            ):
                def qk_projection(xdram, wdram, bias_t, dest_pair):
                    xt = []
                    for i in range(8):
                        t = xp.tile([128, QW], f32r, tag=f"x{i}")
                        (nc.sync if i % 2 == 0 else nc.scalar).dma_start(
                            out=t[:], in_=xdram[i]
                        )
                        xt.append(t)
                    for fh in range(2):
                        wt = []
                        for i in range(8):
                            t = wp.tile([128, 512], f32r, tag=f"w{i}")
                            (nc.sync if i % 2 == 1 else nc.scalar).dma_start(
                                out=t[:], in_=wdram[fh, i]
                            )
                            wt.append(t)
                        for f4 in range(4):
                            f = fh * 4 + f4
                            ps = pps.tile([128, QW], f32, tag="proj")
                            for i in range(8):
                                nc.tensor.matmul(
                                    ps[:],
                                    wt[i][:, f4 * 128 : (f4 + 1) * 128],
                                    xt[i][:],
                                    start=(i == 0),
                                    stop=(i == 7),
                                )
                            # scatter: psum [f 128, r 512] -> pair tiles, strided
                            for c2 in range(2):
                                chunk = 2 * f + c2
                                src_half = ps[c2 * 64 : (c2 + 1) * 64, :]
                                bias_ap = bias_t[c2 * 64 : (c2 + 1) * 64, chunk : chunk + 1]
                                for h in range(4):
                                    dst = (
                                        dest_pair[h // 2][
                                            (h % 2) * 64 : (h % 2) * 64 + 64, :
                                        ]
                                        .rearrange("p (r c) -> p r c", c=16)[:, :, chunk]
                                    )
                                    if h < 3:
                                        nc.vector.tensor_scalar(
                                            out=dst,
                                            in0=src_half[:, h * 128 : (h + 1) * 128],
                                            scalar1=bias_ap,
                                            scalar2=None,
                                            op0=ADD,
                                        )
                                    else:
                                        nc.scalar.activation(
                                            dst,
                                            src_half[:, h * 128 : (h + 1) * 128],
                                            Identity,
                                            bias=bias_ap,
                                        )

                qk_projection(xq, wq, bq_t, qpair)
                qk_projection(xk, wk, bk_t, kpair)

                # ---- v projection (row-major) + reshape + ones col ----
                xt = []
                for i in range(8):
                    t = xp.tile([128, QW], f32r, tag=f"x{i}")
                    (nc.sync if i % 2 == 0 else nc.scalar).dma_start(
                        out=t[:], in_=xv[i]
                    )
                    xt.append(t)
                vstage = [
                    vsb.tile([128, 1024], f32r, tag=f"vst_{rt}", name=f"vst_{rt}")
                    for rt in range(4)
                ]
                vwt = {}
                for fh in range(2):
                    for i in range(8):
                        t = wp.tile([128, 512], f32r, tag=f"w{i}")
                        (nc.sync if i % 2 == 1 else nc.scalar).dma_start(
                            out=t[:], in_=wv[fh, i]
                        )
                        vwt[(fh, i)] = t

                def emit_v_group(rt, fh, pool):
                    ps = pool.tile([128, 512], f32, tag="vproj", name="vproj")
                    for i in range(8):
                        nc.tensor.matmul(
                            ps[:],
                            xt_v[i][:, rt * 128 : (rt + 1) * 128],
                            vwt[(fh, i)][:],
                            start=(i == 0),
                            stop=False,
                        )
                    nc.tensor.matmul(
                        ps[:],
                        ones_row[:],
                        bv_t[:, fh * 512 : (fh + 1) * 512],
                        start=False,
                        stop=True,
                    )
                    nc.vector.tensor_copy(
                        out=vstage[rt][:, fh * 512 : (fh + 1) * 512], in_=ps[:]
                    )

                def emit_v_reshape(rt):
                    for j in range(16):
                        s_ap = vstage[rt][j * 8 : (j + 1) * 8, :].rearrange(
                            "r (c d) -> r c d", d=64
                        )
                        eng = nc.sync if (j % 2 == 0) else nc.scalar
                        eng.dma_start(
                            out=vh[rt][:, j * 65 : j * 65 + 64], in_=s_ap
                        )
                    ones_dst = vh[rt][:].rearrange("p (j e) -> p j e", e=65)[:, :, 64]
                    nc.vector.tensor_scalar(
                        out=ones_dst,
                        in0=vh[rt][:, 0:16],
                        scalar1=0.0,
                        scalar2=1.0,
                        op0=MUL,
                        op1=ADD,
                    )

                xt_v = xt
                # pair-A heads now; pair-B heads are interleaved into the
                # attention phase below (PE filler under ACT-bound exp)
                for rt in range(2 if ivl else 4):
                    for fh in range(2):
                        emit_v_group(rt, fh, pps)
                    emit_v_reshape(rt)

            # ---------------- Phase 2: attention ----------------
            if phases < 2:
                nc.sync.dma_start(out=out[0:128, :].bitcast(f32r), in_=qpair[0][:, 0:1024])
                continue
            with (
                tc.tile_pool(name=f"scps{rep}", bufs=2, space="PSUM") as scps,
                tc.tile_pool(name=f"ctxps{rep}", bufs=2, space="PSUM") as ctxps,
                tc.tile_pool(name=f"vpps{rep}", bufs=1, space="PSUM") as vpps,
            ):
                fillers = []
                if ivl:
                    for rt in range(2, 4):
                        for fh in range(2):
                            fillers.append(lambda rt=rt, fh=fh: emit_v_group(rt, fh, vpps))
                        fillers.append(lambda rt=rt: emit_v_reshape(rt))

                # out-projection weights + interleaved out-proj s-tiles:
                # window qi's 4 s-tiles run as soon as both pairs' ctxT for
                # that window are normalized, on the spare vproj psum bank
                wo_t = []
                for pair in range(2):
                    t = wop.tile([128, 1024], f32r, tag=f"wo{pair}", name=f"wo{pair}")
                    (nc.sync if pair == 0 else nc.scalar).dma_start(
                        out=t[:], in_=wo[pair]
                    )
                    wo_t.append(t)
                emitted_st = set()

                def emit_out_stile(st, on_act=False):
                    emitted_st.add(st)
                    ostage = osb.tile([128, 1024], f32, tag="ostage", name="ostage")
                    for og in range(2):
                        ps = vpps.tile([128, 512], f32, tag="vproj", name="vproj")
                        for pair in range(2):
                            nc.tensor.matmul(
                                ps[:],
                                ctxT[pair][:, st * 128 : (st + 1) * 128],
                                wo_t[pair][:, og * 512 : (og + 1) * 512],
                                start=(pair == 0),
                                stop=(pair == 1),
                            )
                        if on_act:
                            nc.scalar.activation(
                                ostage[:, og * 512 : (og + 1) * 512], ps[:], Copy
                            )
                        else:
                            nc.vector.tensor_copy(
                                out=ostage[:, og * 512 : (og + 1) * 512], in_=ps[:]
                            )
                    (nc.sync if st % 2 == 0 else nc.scalar).dma_start(
                        out=out[st * 128 : (st + 1) * 128, :], in_=ostage[:]
                    )
                def scores(pair, qi, kj):
                    # one [128, 1024] psum duo = both heads' scores for kj
                    sp = scps.tile([128, 2 * QW], f32, tag="sduo")
                    for h2 in range(2):
                        nc.tensor.matmul(
                            sp[:, h2 * QW : (h2 + 1) * QW],
                            kpair[pair][h2 * 64 : h2 * 64 + 64, kj * KT : (kj + 1) * KT],
                            qpair[pair][h2 * 64 : h2 * 64 + 64, qi * QW : (qi + 1) * QW],
                            start=True,
                            stop=True,
                        )
                    return sp

                # flat software-pipelined stream over (qi, pair, kj):
                # scores run one step ahead of exp/mask/ctx across all
                # (pair, qi) boundaries so the PE never waits on ACT.
                steps = []
                for pair in range(2):
                    for qi in range(4):
                        nkt = 4 * qi + 4
                        for kj in range(nkt):
                            steps.append((qi, pair, kj, nkt))

                cps_map = {}
                s_cur = scores(steps[0][1], steps[0][0], steps[0][2])
                for si, (qi, pair, kj, nkt) in enumerate(steps):
                    if si % 2 == 1 and fillers:
                        fillers.pop(0)()
                    if si + 1 < len(steps):
                        nqi, npair, nkj, _ = steps[si + 1]
                        s_next = scores(npair, nqi, nkj)
                    else:
                        s_next = None
                    if kj == 0:
                        cps_map[(qi, pair)] = [
                            ctxps.tile([DK + 1, QW], f32, tag=f"ctx{h2}",
                                       name=f"ctx{h2}", bufs=(2 if h2 == 0 else 1))
                            for h2 in range(2)
                        ]
                    cps = cps_map[(qi, pair)]
                    d = kj - 4 * qi
                    pt = ptp.tile([128, 2 * QW], f32r, tag="ptduo")
                    s3 = s_cur[:].rearrange("p (h x) -> p h x", h=2)
                    p3 = pt[:].rearrange("p (h x) -> p h x", h=2)
                    if d >= 0:
                        if d > 0:
                            nc.vector.tensor_scalar(
                                out=p3[:, :, 0 : 128 * d],
                                in0=s3[:, :, 0 : 128 * d],
                                scalar1=0.0,
                                scalar2=None,
                                op0=MUL,
                            )
                        nc.scalar.activation(
                            p3[:, :, 128 * d :], s3[:, :, 128 * d :], Exp
                        )
                        for h2 in range(2):
                            nc.vector.tensor_tensor(
                                out=pt[:, h2 * QW + 128 * d : h2 * QW + 128 * (d + 1)],
                                in0=pt[:, h2 * QW + 128 * d : h2 * QW + 128 * (d + 1)],
                                in1=tri01[:],
                                op=MUL,
                            )
                    else:
                        nc.scalar.activation(pt[:], s_cur[:], Exp)
                    for h2 in range(2):
                        h = 2 * pair + h2
                        nc.tensor.matmul(
                            cps[h2][:],
                            vh[h][:, kj * 65 : kj * 65 + 65],
                            pt[:, h2 * QW : (h2 + 1) * QW],
                            start=(kj == 0),
                            stop=(kj == nkt - 1),
                        )
                    if kj == nkt - 1:
                        # normalize: ctxU / denom -> ctxT
                        for h2 in range(2):
                            rec = small.tile([1, QW], f32, tag="rec")
                            nc.vector.reciprocal(rec[:], cps[h2][64:65, :])
                            bc = small.tile([64, QW], f32, tag="bc")
                            nc.gpsimd.partition_broadcast(bc[:], rec[:], channels=64)
                            nc.vector.tensor_tensor(
                                out=ctxT[pair][
                                    h2 * 64 : h2 * 64 + 64, qi * QW : (qi + 1) * QW
                                ],
                                in0=cps[h2][0:64, :],
                                in1=bc[:],
                                op=MUL,
                            )
                        del cps_map[(qi, pair)]
                        if pair == 1 and qi < 3:
                            for st in range(qi * 4, qi * 4 + 4):
                                fillers.append(
                                    lambda st=st: emit_out_stile(st, on_act=False)
                                )
                    s_cur = s_next
                # flush any fillers that did not get a pop slot
                while fillers:
                    fillers.pop(0)()

            # ---------------- Phase 3: output projection ----------------
            if phases < 3:
                nc.sync.dma_start(out=out[0:128, :].bitcast(f32r), in_=ctxT[0][:, 0:1024])
                continue
            with (
                tc.tile_pool(name=f"ops{rep}", bufs=3, space="PSUM") as ops,
            ):
                for st in range(16):
                    if st in emitted_st:
                        continue
                    ostage = osb.tile([128, 1024], f32, tag="ostage")
                    for og in range(2):
                        ps = ops.tile([128, 512], f32, tag="ops")
                        for pair in range(2):
                            nc.tensor.matmul(
                                ps[:],
                                ctxT[pair][:, st * 128 : (st + 1) * 128],
                                wo_t[pair][:, og * 512 : (og + 1) * 512],
                                start=(pair == 0),
                                stop=(pair == 1),
                            )
                        nc.scalar.activation(
                            ostage[:, og * 512 : (og + 1) * 512], ps[:], Copy
                        )
                    (nc.sync if st % 2 == 0 else nc.scalar).dma_start(
                        out=out[st * 128 : (st + 1) * 128, :], in_=ostage[:]
                    )

    nc.finalize()
    return nc


_NC_CACHE = {}


def _get_program(repeat=1, phases=3, ivl=True):
    key = (repeat, phases, ivl)
    if key not in _NC_CACHE:
        _NC_CACHE[key] = _build_program(repeat, phases, ivl)
    return _NC_CACHE[key]


def _host_inputs(query, key, value, Wq, bq, Wk, bk, Wv, bv, Wo):
    """Build the 8 per-core input maps (numpy, host-side shard/transpose)."""
    query = np.asarray(query, dtype=np.float32)
    key = np.asarray(key, dtype=np.float32)
    value = np.asarray(value, dtype=np.float32)
    Wq = np.asarray(Wq, dtype=np.float32)
    Wk = np.asarray(Wk, dtype=np.float32)
    Wv = np.asarray(Wv, dtype=np.float32)
    Wo = np.asarray(Wo, dtype=np.float32)
    bq = np.asarray(bq, dtype=np.float32)
    bk = np.asarray(bk, dtype=np.float32)
    bv = np.asarray(bv, dtype=np.float32)

    scale = 1.0 / np.sqrt(np.float32(DK))

    def round_f32r(a):
        # round-to-nearest-even into the fp32r format (11 mantissa bits,
        # low 12 bits zero) expected by fp32r matmul operands
        b = np.ascontiguousarray(a, dtype=np.float32).view(np.uint32)
        r = (b + np.uint32(0x7FF) + ((b >> np.uint32(12)) & np.uint32(1))) & np.uint32(
            0xFFFFF000
        )
        return r.view(np.float32)

    def wtile(WT):  # [1024 i,1024 f] -> [2, 8, 128, 512] (f-half, i-tile)
        return round_f32r(
            np.ascontiguousarray(WT.reshape(8, 128, 2, 512).transpose(2, 0, 1, 3))
        )

    wq4 = wtile(Wq.T * scale)
    wk4 = wtile(Wk.T)
    wv4 = wtile(Wv.T)
    WoT = np.ascontiguousarray(Wo.T)  # [i, o]

    def dup_bias(b):  # [1024] -> [128, 16] dup layout
        m = b.reshape(16, 64).T  # [64, 16]
        return np.ascontiguousarray(np.vstack([m, m]))

    bqd = dup_bias(bq * scale)
    bkd = dup_bias(bk)
    bvr = round_f32r(bv.reshape(1, 1024))
    tri01 = np.ascontiguousarray(np.triu(np.ones((128, 128), np.float32)))

    in_maps = []
    for core in range(N_CORES):
        b, g = divmod(core, 4)
        sl = slice(g * ROWS_PER_CORE, (g + 1) * ROWS_PER_CORE)
        xq = round_f32r(query[b, sl, :].T).reshape(8, 128, QW)
        xk = round_f32r(key[b, sl, :].T).reshape(8, 128, QW)
        xv = round_f32r(value[b, sl, :].T).reshape(8, 128, QW)
        wo4 = round_f32r(WoT[g * 256 : (g + 1) * 256, :]).reshape(2, 128, 1024)
        in_maps.append(
            {
                "ones128": np.ones((1, 128), np.float32),
                "xq": xq,
                "xk": xk,
                "xv": xv,
                "wq": wq4,
                "wk": wk4,
                "wv": wv4,
                "wo": wo4,
                "bqd": bqd,
                "bkd": bkd,
                "bvr": bvr,
                "tri": tri01,
            }
        )
    return in_maps


def run_cores(in_maps, trace=False, trace_kwargs=None, repeat=1):
    """Compile + run the SPMD program on cores 0-7, return BassKernelResults."""
    from concourse.bass_utils import run_bass_kernel_spmd

    nc = _get_program(repeat)
    kwargs = {}
    if trace:
        kwargs["trace"] = True
        if trace_kwargs:
            kwargs["trace_kwargs"] = trace_kwargs
    return run_bass_kernel_spmd(nc, in_maps, core_ids=list(range(N_CORES)), **kwargs)


def kernel(query, key, value, mask, Wq, bq, Wk, bk, Wv, bv, Wo, bo, _trace=False):
    in_maps = _host_inputs(query, key, value, Wq, bq, Wk, bk, Wv, bv, Wo)
    res = run_cores(in_maps, trace=_trace)
    bo = np.asarray(bo, dtype=np.float32)
    out = np.zeros((B, S, D), dtype=np.float32)
    for core in range(N_CORES):
        b = core // 4
        out[b] += res.results[core]["out"]
    out += bo[None, None, :]
    kernel.last_results = res
    return out



# revision 2
# speedup vs baseline: 1.1310x; 1.1310x over previous
"""Trainium2 Bass kernel for nn_MultiHeadAttention (B=2, S=2048, D=1024, H=16).

Reference semantics (note the *raw-view* head split):
    q = query @ Wq.T + bq                  # [B, S, D]
    q = q.reshape(B, H, S, DK)             # raw view: head h = rows [h*128,(h+1)*128) of q[b]
                                           #   viewed as [2048, 64]
    scores = q @ k.T / sqrt(DK), causal mask, softmax
    ctx    = softmax @ v                   # [B, H, S, DK]
    out    = ctx.transpose(0,2,1,3).reshape(B,S,D) @ Wo.T + bo

Sharding: 8 cores = 2 batches x 4 head-groups.  Core (b, g) owns heads
[4g, 4g+4) of batch b, i.e. rows [512g, 512g+512) of the QKV projections.
Each core computes its 4 heads' attention plus its partial contribution
C_heads @ Wo[:, head cols].T of the output projection; the host sums the 4
partials per batch and adds bo.

v2 layout (bf16 compute, fp32 psum accumulate):
  - x and W tiles stream in bf16 (halves HBM traffic vs fp32).
  - q,k projections emit feature-major psum tiles [f,r]; a strided scatter
    builds per-head-pair bf16 tensors qT/kT [128=2 heads x 64 dk, 2048 pos].
  - v projection emits row-major [r,f]; a DMA reshape builds bf16 vh tiles
    [128 pos, 64 dk] (+ ones column for softmax denominators).
  - scores_T = kT.T @ qT per (k-tile 128, q-window 512); on the causal
    diagonal only cols >= 128*d are computed (partial-N matmuls) so no
    zeroing pass is needed; exp on ACT writes bf16 pt; the 128-wide
    boundary block is masked by a tri01 multiply on DVE (bf16, 2x rate).
  - ctx_T[dk(+denom), q] accumulates with M=65 matmuls (vh ones column ->
    row 64 = softmax denominator), streaming only the live suffix.
  - normalize with reciprocal + gpsimd partition_broadcast, then the output
    projection out[s, o] = sum_pairs ctxT_pair.T @ WoT_pair, emitted fp16.
"""

import os
import sys

import numpy as np

_TRN_REPO = "/opt/trn_rl_repo"
if _TRN_REPO not in sys.path:
    sys.path.insert(0, _TRN_REPO)

B, S, D, H = 2, 2048, 1024, 16
DK = D // H  # 64
N_CORES = 8
HEADS_PER_CORE = H // 4  # 4
ROWS_PER_CORE = HEADS_PER_CORE * (S // H)  # 512 rows of the projection output
QW = 512  # q-position window (psum free-dim)
KT = 128  # k-position tile


def _build_program(repeat=1, phases=3, ivl=True):
    import concourse.bass as bass
    import concourse.bacc as bacc
    import concourse.mybir as mybir
    from concourse.tile import TileContext
    from concourse import library_config

    f32 = mybir.dt.float32
    bf16 = mybir.dt.bfloat16
    f16 = mybir.dt.float16
    Exp = mybir.ActivationFunctionType.Exp
    Copy = mybir.ActivationFunctionType.Copy
    Identity = mybir.ActivationFunctionType.Identity
    MUL = mybir.AluOpType.mult
    ADD = mybir.AluOpType.add

    nc = bacc.Bacc("TRN2", target_bir_lowering=False, debug=False)

    # ---- DRAM parameters (host pre-tiled / pre-transposed) ----
    xq = nc.dram_tensor("xq", [8, 128, QW], bf16, kind="ExternalInput")
    xk = nc.dram_tensor("xk", [8, 128, QW], bf16, kind="ExternalInput")
    xv = nc.dram_tensor("xv", [8, 128, QW], bf16, kind="ExternalInput")
    wq = nc.dram_tensor("wq", [2, 8, 128, 512], bf16, kind="ExternalInput")
    wk = nc.dram_tensor("wk", [2, 8, 128, 512], bf16, kind="ExternalInput")
    wv = nc.dram_tensor("wv", [2, 8, 128, 512], bf16, kind="ExternalInput")
    wo = nc.dram_tensor("wo", [2, 128, 1024], bf16, kind="ExternalInput")
    bqd = nc.dram_tensor("bqd", [128, 16], f32, kind="ExternalInput")
    bkd = nc.dram_tensor("bkd", [128, 16], f32, kind="ExternalInput")
    bvr = nc.dram_tensor("bvr", [1, 1024], bf16, kind="ExternalInput")
    tri = nc.dram_tensor("tri", [128, 128], bf16, kind="ExternalInput")
    ones128 = nc.dram_tensor("ones128", [1, 128], bf16, kind="ExternalInput")
    out = nc.dram_tensor("out", [S, D], f16, kind="ExternalOutput")

    with TileContext(nc) as tc:
      from contextlib import ExitStack
      with ExitStack() as stack:
        persist = stack.enter_context(tc.tile_pool(name="persist", bufs=1))
        vhp = stack.enter_context(tc.tile_pool(name="vhp", bufs=1))
        small = stack.enter_context(tc.tile_pool(name="small", bufs=4))
        xp = stack.enter_context(tc.tile_pool(name="xp", bufs=2))
        wp = stack.enter_context(tc.tile_pool(name="wp", bufs=2))
        vsb = stack.enter_context(tc.tile_pool(name="vsb", bufs=1))
        ptp = stack.enter_context(tc.tile_pool(name="ptp", bufs=4))
        wop = stack.enter_context(tc.tile_pool(name="wop", bufs=1))
        osb = stack.enter_context(tc.tile_pool(name="osb", bufs=3))
        for rep in range(repeat):
            # persistent tiles
            qpair = [persist.tile([128, S], bf16, tag=f"qpair{p}", name=f"qpair{p}") for p in range(2)]
            kpair = [persist.tile([128, S], bf16, tag=f"kpair{p}", name=f"kpair{p}") for p in range(2)]
            ctxT = [persist.tile([128, S], bf16, tag=f"ctxT{p}", name=f"ctxT{p}") for p in range(2)]
            tri01 = persist.tile([128, 128], bf16, tag="tri01")
            bq_t = persist.tile([128, 16], f32, tag="bq_t")
            bk_t = persist.tile([128, 16], f32, tag="bk_t")
            bv_t = persist.tile([1, 1024], bf16, tag="bv_t")
            ones_row = persist.tile([1, 128], bf16, tag="ones_row")
            nc.sync.dma_start(out=tri01[:], in_=tri[:])
            nc.sync.dma_start(out=bq_t[:], in_=bqd[:])
            nc.sync.dma_start(out=bk_t[:], in_=bkd[:])
            nc.sync.dma_start(out=bv_t[:], in_=bvr[:])
            nc.sync.dma_start(out=ones_row[:], in_=ones128[:])


            # vh: one tile per head [128, 16*(DK+1)]; per ktile j cols
            # [j*65, j*65+64) = v data, col j*65+64 = ones (softmax denom)
            vh = [
                vhp.tile([128, 16 * (DK + 1)], bf16, tag=f"vh_{h}", name=f"vh_{h}")
                for h in range(4)
            ]

            # ---------------- Phase 1: projections ----------------
            with (
                tc.tile_pool(name=f"pps{rep}", bufs=3, space="PSUM") as pps,
            ):
                def qk_projection(xdram, wdram, bias_t, dest_pair):
                    xt = []
                    for i in range(8):
                        t = xp.tile([128, QW], bf16, tag=f"x{i}")
                        (nc.sync if i % 2 == 0 else nc.scalar).dma_start(
                            out=t[:], in_=xdram[i]
                        )
                        xt.append(t)
                    for fh in range(2):
                        wt = []
                        for i in range(8):
                            t = wp.tile([128, 512], bf16, tag=f"w{i}")
                            (nc.sync if i % 2 == 1 else nc.scalar).dma_start(
                                out=t[:], in_=wdram[fh, i]
                            )
                            wt.append(t)
                        for f4 in range(4):
                            f = fh * 4 + f4
                            ps = pps.tile([128, QW], f32, tag="proj")
                            for i in range(8):
                                nc.tensor.matmul(
                                    ps[:],
                                    wt[i][:, f4 * 128 : (f4 + 1) * 128],
                                    xt[i][:],
                                    start=(i == 0),
                                    stop=(i == 7),
                                )
                            # scatter: psum [f 128, r 512] -> pair tiles, strided
                            for c2 in range(2):
                                chunk = 2 * f + c2
                                src_half = ps[c2 * 64 : (c2 + 1) * 64, :]
                                bias_ap = bias_t[c2 * 64 : (c2 + 1) * 64, chunk : chunk + 1]
                                for h in range(4):
                                    dst = (
                                        dest_pair[h // 2][
                                            (h % 2) * 64 : (h % 2) * 64 + 64, :
                                        ]
                                        .rearrange("p (r c) -> p r c", c=16)[:, :, chunk]
                                    )
                                    if h < 3:
                                        nc.vector.tensor_scalar(
                                            out=dst,
                                            in0=src_half[:, h * 128 : (h + 1) * 128],
                                            scalar1=bias_ap,
                                            scalar2=None,
                                            op0=ADD,
                                        )
                                    else:
                                        nc.scalar.activation(
                                            dst,
                                            src_half[:, h * 128 : (h + 1) * 128],
                                            Identity,
                                            bias=bias_ap,
                                        )

                qk_projection(xq, wq, bq_t, qpair)
                qk_projection(xk, wk, bk_t, kpair)

                # ---- v projection (row-major) + reshape + ones col ----
                xt = []
                for i in range(8):
                    t = xp.tile([128, QW], bf16, tag=f"x{i}")
                    (nc.sync if i % 2 == 0 else nc.scalar).dma_start(
                        out=t[:], in_=xv[i]
                    )
                    xt.append(t)
                vstage = [
                    vsb.tile([128, 1024], bf16, tag=f"vst_{rt}", name=f"vst_{rt}")
                    for rt in range(4)
                ]
                vwt = {}
                for fh in range(2):
                    for i in range(8):
                        t = wp.tile([128, 512], bf16, tag=f"w{i}")
                        (nc.sync if i % 2 == 1 else nc.scalar).dma_start(
                            out=t[:], in_=wv[fh, i]
                        )
                        vwt[(fh, i)] = t

                def emit_v_group(rt, fh, pool):
                    ps = pool.tile([128, 512], f32, tag="vproj", name="vproj")
                    for i in range(8):
                        nc.tensor.matmul(
                            ps[:],
                            xt_v[i][:, rt * 128 : (rt + 1) * 128],
                            vwt[(fh, i)][:],
                            start=(i == 0),
                            stop=False,
                        )
                    nc.tensor.matmul(
                        ps[:],
                        ones_row[:],
                        bv_t[:, fh * 512 : (fh + 1) * 512],
                        start=False,
                        stop=True,
                    )
                    nc.vector.tensor_copy(
                        out=vstage[rt][:, fh * 512 : (fh + 1) * 512], in_=ps[:]
                    )

                def emit_v_reshape(rt):
                    for j in range(16):
                        s_ap = vstage[rt][j * 8 : (j + 1) * 8, :].rearrange(
                            "r (c d) -> r c d", d=64
                        )
                        eng = nc.sync if (j % 2 == 0) else nc.scalar
                        eng.dma_start(
                            out=vh[rt][:, j * 65 : j * 65 + 64], in_=s_ap
                        )
                    ones_dst = vh[rt][:].rearrange("p (j e) -> p j e", e=65)[:, :, 64]
                    nc.vector.tensor_scalar(
                        out=ones_dst,
                        in0=vh[rt][:, 0:16],
                        scalar1=0.0,
                        scalar2=1.0,
                        op0=MUL,
                        op1=ADD,
                    )

                xt_v = xt
                # pair-A heads now; pair-B heads are interleaved into the
                # attention phase below (PE filler under ACT-bound exp)
                for rt in range(2 if ivl else 4):
                    for fh in range(2):
                        emit_v_group(rt, fh, pps)
                    emit_v_reshape(rt)

            # ---------------- Phase 2: attention ----------------
            with (
                tc.tile_pool(name=f"scps{rep}", bufs=2, space="PSUM") as scps,
                tc.tile_pool(name=f"ctxps{rep}", bufs=2, space="PSUM") as ctxps,
                tc.tile_pool(name=f"vpps{rep}", bufs=1, space="PSUM") as vpps,
            ):
                fillers = []
                if ivl:
                    for rt in range(2, 4):
                        for fh in range(2):
                            fillers.append(lambda rt=rt, fh=fh: emit_v_group(rt, fh, vpps))
                        fillers.append(lambda rt=rt: emit_v_reshape(rt))

                # out-projection weights + interleaved out-proj s-tiles:
                # window qi's 4 s-tiles run as soon as both pairs' ctxT for
                # that window are normalized, on the spare vproj psum bank
                wo_t = []
                for pair in range(2):
                    t = wop.tile([128, 1024], bf16, tag=f"wo{pair}", name=f"wo{pair}")
                    (nc.sync if pair == 0 else nc.scalar).dma_start(
                        out=t[:], in_=wo[pair]
                    )
                    wo_t.append(t)
                emitted_st = set()

                def emit_out_stile(st, on_act=False):
                    emitted_st.add(st)
                    ostage = osb.tile([128, 1024], f16, tag="ostage", name="ostage")
                    for og in range(2):
                        ps = vpps.tile([128, 512], f32, tag="vproj", name="vproj")
                        for pair in range(2):
                            nc.tensor.matmul(
                                ps[:],
                                ctxT[pair][:, st * 128 : (st + 1) * 128],
                                wo_t[pair][:, og * 512 : (og + 1) * 512],
                                start=(pair == 0),
                                stop=(pair == 1),
                            )
                        if on_act:
                            nc.scalar.activation(
                                ostage[:, og * 512 : (og + 1) * 512], ps[:], Copy
                            )
                        else:
                            nc.vector.tensor_copy(
                                out=ostage[:, og * 512 : (og + 1) * 512], in_=ps[:]
                            )
                    (nc.sync if st % 2 == 0 else nc.scalar).dma_start(
                        out=out[st * 128 : (st + 1) * 128, :], in_=ostage[:]
                    )
                def scores(pair, qi, kj):
                    # one [128, 1024] psum duo = both heads' scores for kj;
                    # on the causal diagonal only cols >= 128*d are computed
                    d = kj - 4 * qi
                    off = 128 * d if d > 0 else 0
                    sp = scps.tile([128, 2 * QW], f32, tag="sduo")
                    for h2 in range(2):
                        nc.tensor.matmul(
                            sp[:, h2 * QW + off : (h2 + 1) * QW],
                            kpair[pair][h2 * 64 : h2 * 64 + 64, kj * KT : (kj + 1) * KT],
                            qpair[pair][h2 * 64 : h2 * 64 + 64, qi * QW + off : (qi + 1) * QW],
                            start=True,
                            stop=True,
                        )
                    return sp

                # flat software-pipelined stream over (qi, pair, kj):
                # scores run one step ahead of exp/mask/ctx across all
                # (pair, qi) boundaries so the PE never waits on ACT.
                steps = []
                for pair in range(2):
                    for qi in range(4):
                        nkt = 4 * qi + 4
                        for kj in range(nkt):
                            steps.append((qi, pair, kj, nkt))

                cps_map = {}
                s_cur = scores(steps[0][1], steps[0][0], steps[0][2])
                for si, (qi, pair, kj, nkt) in enumerate(steps):
                    if si % 2 == 1 and fillers:
                        fillers.pop(0)()
                    if si + 1 < len(steps):
                        nqi, npair, nkj, _ = steps[si + 1]
                        s_next = scores(npair, nqi, nkj)
                    else:
                        s_next = None
                    if kj == 0:
                        cps_map[(qi, pair)] = [
                            ctxps.tile([DK + 1, QW], f32, tag=f"ctx{h2}",
                                       name=f"ctx{h2}", bufs=(2 if h2 == 0 else 1))
                            for h2 in range(2)
                        ]
                    cps = cps_map[(qi, pair)]
                    d = kj - 4 * qi
                    off = 128 * d if d > 0 else 0
                    pt = ptp.tile([128, 2 * QW], bf16, tag="ptduo")
                    s3 = s_cur[:].rearrange("p (h x) -> p h x", h=2)
                    p3 = pt[:].rearrange("p (h x) -> p h x", h=2)
                    if d >= 0:
                        nc.scalar.activation(
                            p3[:, :, off:], s3[:, :, off:], Exp
                        )
                        for h2 in range(2):
                            nc.vector.tensor_tensor(
                                out=pt[:, h2 * QW + off : h2 * QW + off + 128],
                                in0=pt[:, h2 * QW + off : h2 * QW + off + 128],
                                in1=tri01[:],
                                op=MUL,
                            )
                    else:
                        nc.scalar.activation(pt[:], s_cur[:], Exp)
                    for h2 in range(2):
                        h = 2 * pair + h2
                        nc.tensor.matmul(
                            cps[h2][:, off:],
                            vh[h][:, kj * 65 : kj * 65 + 65],
                            pt[:, h2 * QW + off : (h2 + 1) * QW],
                            start=(kj == 0),
                            stop=(kj == nkt - 1),
                        )
                    if kj == nkt - 1:
                        # normalize: ctxU / denom -> ctxT
                        for h2 in range(2):
                            rec = small.tile([1, QW], f32, tag="rec")
                            nc.vector.reciprocal(rec[:], cps[h2][64:65, :])
                            bc = small.tile([64, QW], f32, tag="bc")
                            nc.gpsimd.partition_broadcast(bc[:], rec[:], channels=64)
                            nc.vector.tensor_tensor(
                                out=ctxT[pair][
                                    h2 * 64 : h2 * 64 + 64, qi * QW : (qi + 1) * QW
                                ],
                                in0=cps[h2][0:64, :],
                                in1=bc[:],
                                op=MUL,
                            )
                        del cps_map[(qi, pair)]
                        if pair == 1 and qi < 3:
                            for st in range(qi * 4, qi * 4 + 4):
                                fillers.append(
                                    lambda st=st: emit_out_stile(st, on_act=False)
                                )
                    s_cur = s_next
                # flush any fillers that did not get a pop slot
                while fillers:
                    fillers.pop(0)()

            # ---------------- Phase 3: output projection ----------------
            with (
                tc.tile_pool(name=f"ops{rep}", bufs=3, space="PSUM") as ops,
            ):
                for st in range(16):
                    if st in emitted_st:
                        continue
                    ostage = osb.tile([128, 1024], f16, tag="ostage")
                    for og in range(2):
                        ps = ops.tile([128, 512], f32, tag="ops")
                        for pair in range(2):
                            nc.tensor.matmul(
                                ps[:],
                                ctxT[pair][:, st * 128 : (st + 1) * 128],
                                wo_t[pair][:, og * 512 : (og + 1) * 512],
                                start=(pair == 0),
                                stop=(pair == 1),
                            )
                        nc.scalar.activation(
                            ostage[:, og * 512 : (og + 1) * 512], ps[:], Copy
                        )
                    (nc.sync if st % 2 == 0 else nc.scalar).dma_start(
                        out=out[st * 128 : (st + 1) * 128, :], in_=ostage[:]
                    )

    nc.finalize()
    return nc


_NC_CACHE = {}


def _get_program(repeat=1, phases=3, ivl=True):
    key = (repeat, phases, ivl)
    if key not in _NC_CACHE:
        _NC_CACHE[key] = _build_program(repeat, phases, ivl)
    return _NC_CACHE[key]


def _host_inputs(query, key, value, Wq, bq, Wk, bk, Wv, bv, Wo):
    """Build the 8 per-core input maps (numpy, host-side shard/transpose)."""
    import ml_dtypes

    bf16 = ml_dtypes.bfloat16
    query = np.asarray(query, dtype=np.float32)
    key = np.asarray(key, dtype=np.float32)
    value = np.asarray(value, dtype=np.float32)
    Wq = np.asarray(Wq, dtype=np.float32)
    Wk = np.asarray(Wk, dtype=np.float32)
    Wv = np.asarray(Wv, dtype=np.float32)
    Wo = np.asarray(Wo, dtype=np.float32)
    bq = np.asarray(bq, dtype=np.float32)
    bk = np.asarray(bk, dtype=np.float32)
    bv = np.asarray(bv, dtype=np.float32)

    scale = 1.0 / np.sqrt(np.float32(DK))

    def wtile(WT):  # [1024 i,1024 f] -> [2, 8, 128, 512] (f-half, i-tile)
        return np.ascontiguousarray(
            WT.reshape(8, 128, 2, 512).transpose(2, 0, 1, 3)
        ).astype(bf16)

    wq4 = wtile(Wq.T * scale)
    wk4 = wtile(Wk.T)
    wv4 = wtile(Wv.T)
    WoT = np.ascontiguousarray(Wo.T)  # [i, o]

    def dup_bias(b):  # [1024] -> [128, 16] dup layout
        m = b.reshape(16, 64).T  # [64, 16]
        return np.ascontiguousarray(np.vstack([m, m]))

    bqd = dup_bias(bq * scale)
    bkd = dup_bias(bk)
    bvr = bv.reshape(1, 1024).astype(bf16)
    tri01 = np.ascontiguousarray(np.triu(np.ones((128, 128), np.float32))).astype(bf16)

    in_maps = []
    for core in range(N_CORES):
        b, g = divmod(core, 4)
        sl = slice(g * ROWS_PER_CORE, (g + 1) * ROWS_PER_CORE)
        xq = np.ascontiguousarray(query[b, sl, :].T).astype(bf16).reshape(8, 128, QW)
        xk = np.ascontiguousarray(key[b, sl, :].T).astype(bf16).reshape(8, 128, QW)
        xv = np.ascontiguousarray(value[b, sl, :].T).astype(bf16).reshape(8, 128, QW)
        wo4 = np.ascontiguousarray(
            WoT[g * 256 : (g + 1) * 256, :]
        ).astype(bf16).reshape(2, 128, 1024)
        in_maps.append(
            {
                "ones128": np.ones((1, 128), bf16),
                "xq": xq,
                "xk": xk,
                "xv": xv,
                "wq": wq4,
                "wk": wk4,
                "wv": wv4,
                "wo": wo4,
                "bqd": bqd,
                "bkd": bkd,
                "bvr": bvr,
                "tri": tri01,
            }
        )
    return in_maps


def run_cores(in_maps, trace=False, trace_kwargs=None, repeat=1):
    """Compile + run the SPMD program on cores 0-7, return BassKernelResults."""
    from concourse.bass_utils import run_bass_kernel_spmd

    nc = _get_program(repeat)
    kwargs = {}
    if trace:
        kwargs["trace"] = True
        if trace_kwargs:
            kwargs["trace_kwargs"] = trace_kwargs
    return run_bass_kernel_spmd(nc, in_maps, core_ids=list(range(N_CORES)), **kwargs)


def kernel(query, key, value, mask, Wq, bq, Wk, bk, Wv, bv, Wo, bo, _trace=False):
    in_maps = _host_inputs(query, key, value, Wq, bq, Wk, bk, Wv, bv, Wo)
    res = run_cores(in_maps, trace=_trace)
    bo = np.asarray(bo, dtype=np.float32)
    out = np.zeros((B, S, D), dtype=np.float32)
    for core in range(N_CORES):
        b = core // 4
        out[b] += res.results[core]["out"].astype(np.float32)
    out += bo[None, None, :]
    kernel.last_results = res
    return out
